# revision 1
# baseline (speedup 1.0000x reference)
"""Trainium2 Bass kernel for nn_ExportPreQuantizedLayer.

Computes: out = fake_quant(x) @ dequant(weight_q).T + bias
  x_q  = clip(round_half_away(x / a_scale) + a_zp, 0, 255)
  x_dq = (x_q - a_zp) * a_scale
  W    = (weight_q - w_zp[:, None]) * w_scale[:, None]      # [out, in]
  out  = einsum('bsk,ok->bso', x_dq, W) + bias

Sharding: 2D grid over the 8 cores — 4 shards of out_features (O) x
2 shards of tokens (N).  Each core computes a [2048, 2048] block of
out^T.  Key algebra: with xi = clip(round(x/s), -z, 255-z) and
wi = wq - wzp (both exact small integers, representable in bf16),

  out[o, n] = s * ws[o] * (wi @ xi^T)[o, n] + bias[o]

so the matmul runs at full bf16 PE rate and the epilogue is a single
per-partition (per-o) scale+bias on the scalar engine.

Rounding: round-to-nearest on the device is done with the fp32
magic-number trick t = x*(1/s) + 1.5*2^23 (RNE); clip is applied in the
shifted domain and the magic constant is subtracted in the final pass
that also casts to bf16.  This differs from the reference's
round-half-away-from-zero only on exact .5 ties (measure-zero for
random float inputs).
"""

import sys

if "/opt/trn_rl_repo" not in sys.path:
    sys.path.insert(0, "/opt/trn_rl_repo")

import ml_dtypes
import numpy as np

import concourse.bass as bass
import concourse.mybir as mybir
import concourse.tile as tile
from concourse import bacc
from concourse.bass_utils import run_bass_kernel_spmd

F32 = mybir.dt.float32
BF16 = mybir.dt.bfloat16
U8 = mybir.dt.uint8
MAGIC = 12582912.0  # 1.5 * 2**23: fp32 round-to-int magic constant

# Full problem shape (hardcoded per spec)
B, S, DIN, DOUT = 2, 2048, 2048, 8192
N_CORES = 8
O_SPLIT, N_SPLIT = 4, 2  # 4 shards of DOUT x 2 shards of tokens


def build_nc(K, N, O, reps=1, quant="magic", wsub_engine="vector", clip_engine="vector", fine_first=0, dma_split=1):
    """Build the per-core Bass program.

    reps > 1 wraps the whole body in a device-side loop — used only for
    timing (device work scales with reps while dispatch overhead doesn't).

    quant: "magic" = ACT x*rs+2^23 trick + 2 DVE clip passes;
           "cast"  = 2 DVE passes using the HW's round+saturate u8 cast
                     (HW-exact; the CoreSim interpreter models u8 casts as
                     truncate+wrap, so sim disagrees for "cast").
    wsub_engine: engine for the weight-zero-point subtract ("vector" DVE /
           "gpsimd" POOL — POOL is otherwise idle during the lead-in).

    Inputs (per core):
      xT      [K, N]   f32   x^T shard (tokens on the free axis)
      wqT     [K, O]   u8    weight_q^T shard
      wzpb    [128, O] bf16  w_zp broadcast along partitions
      aparams [128, 2] f32   (a_scale, a_zp) broadcast along partitions
      wsc     [128, O//128] f32  w_scale laid out [p, ot] with o = ot*128+p
      biasc   [128, O//128] f32  bias, same layout
    Output:
      out     [O, N]  f32    out^T shard
    """
    KT = K // 128
    OT = O // 128
    NB = N // 512
    AF = mybir.ActivationFunctionType
    OP = mybir.AluOpType

    nc = bacc.Bacc("TRN2", target_bir_lowering=False, debug=False, num_devices=N_CORES)
    xT = nc.declare_dram_parameter("xT", [K, N], F32, isOutput=False)
    wqT = nc.declare_dram_parameter("wqT", [K, O], U8, isOutput=False)
    wzpb = nc.declare_dram_parameter("wzpb", [128, O], BF16, isOutput=False)
    aparams = nc.declare_dram_parameter("aparams", [128, 2], F32, isOutput=False)
    wsc = nc.declare_dram_parameter("wsc", [128, OT], F32, isOutput=False)
    biasc = nc.declare_dram_parameter("biasc", [128, OT], F32, isOutput=False)
    out = nc.declare_dram_parameter("out", [O, N], F32, isOutput=True)

    with tile.TileContext(nc) as tc:
        with (
            tc.tile_pool(name="const", bufs=1) as cpool,
            tc.tile_pool(name="big", bufs=1) as bigpool,
            tc.tile_pool(name="xin", bufs=2) as xpool,
            tc.tile_pool(name="tq", bufs=2) as tpool,
            tc.tile_pool(name="win", bufs=2) as wpool,
            tc.tile_pool(name="oout", bufs=6) as opool,
            tc.tile_pool(name="psum", bufs=8, space="PSUM") as psum_pool,
        ):
            def body():
                _kernel_body(
                    nc, tc, KT, OT, NB, N, O,
                    xT, wqT, wzpb, aparams, wsc, biasc, out,
                    cpool, bigpool, xpool, tpool, wpool, opool, psum_pool,
                    quant, wsub_engine, clip_engine, fine_first, dma_split,
                )

            if reps > 1:
                with tc.For_i(0, reps, 1):
                    body()
            else:
                body()

    nc.compile()
    return nc


def _kernel_body(
    nc, tc, KT, OT, NB, N, O,
    xT, wqT, wzpb, aparams, wsc, biasc, out,
    cpool, bigpool, xpool, tpool, wpool, opool, psum_pool,
    quant="magic", wsub_engine="vector", clip_engine="vector", fine_first=0,
    dma_split=1,
):
    AF = mybir.ActivationFunctionType
    OP = mybir.AluOpType
    # --- scalar prep -------------------------------------------------
    ap_sb = cpool.tile([128, 2], F32)
    nc.sync.dma_start(ap_sb[:], aparams[:])
    rs = cpool.tile([128, 1], F32)
    nc.vector.reciprocal(rs[:], ap_sb[:, 0:1])
    # loC = MAGIC - z ; hiC = loC + 255  (clip bounds in shifted domain)
    loC = cpool.tile([128, 1], F32)
    nc.vector.tensor_scalar(loC[:], ap_sb[:, 1:2], -1.0, MAGIC, OP.mult, OP.add)
    hiC = cpool.tile([128, 1], F32)
    nc.vector.tensor_scalar_add(hiC[:], loC[:], 255.0)
    magic = cpool.tile([128, 1], F32)
    nc.vector.memset(magic[:], MAGIC)

    ws_sb = cpool.tile([128, OT], F32)
    nc.sync.dma_start(ws_sb[:], wsc[:])
    beta = cpool.tile([128, OT], F32)
    nc.sync.dma_start(beta[:], biasc[:])
    alpha = cpool.tile([128, OT], F32)  # alpha = a_scale * w_scale
    nc.vector.tensor_scalar_mul(alpha[:], ws_sb[:], ap_sb[:, 0:1])

    wzpb_sb = cpool.tile([128, O], BF16)
    nc.sync.dma_start(wzpb_sb[:], wzpb[:])

    # --- streaming quantization of x and weight dequant --------------
    xiT = bigpool.tile([128, KT, N], BF16)
    wiT = bigpool.tile([128, KT, O], BF16)

    for kt in range(KT):
        xf = xpool.tile([128, N], F32)
        # chunk the first tiles so the first matmuls start early; dma_split
        # spreads each x-tile DMA over multiple HWDGE queues
        nch = 4 if kt < fine_first else dma_split
        cw = N // nch
        for ci in range(nch):
            cs = slice(ci * cw, (ci + 1) * cw)
            nc.sync.dma_start(xf[:, cs], xT[kt * 128 : (kt + 1) * 128, cs])
        if quant == "magic":
            # t = x * (1/s) + MAGIC -> fp32 RNE rounds to integer + MAGIC
            t = tpool.tile([128, N], F32)
            for ci in range(nch):
                cs = slice(ci * cw, (ci + 1) * cw)
                nc.scalar.activation(
                    t[:, cs], xf[:, cs], AF.Identity,
                    bias=magic[:, 0:1], scale=rs[:, 0:1]
                )
                # clip-lo in place, then clip-hi and subtract MAGIC -> bf16
                getattr(nc, clip_engine).tensor_scalar_max(
                    t[:, cs], t[:, cs], loC[:, 0:1]
                )
                nc.vector.tensor_scalar(
                    xiT[:, kt, cs], t[:, cs], hiC[:, 0:1], -MAGIC, OP.min, OP.add
                )
        else:
            # xq = sat_u8(round(x*rs + z)); xi = xq - z  (HW cast
            # rounds-to-nearest and saturates at [0, 255])
            xq = tpool.tile([128, N], U8)
            nc.vector.tensor_scalar(
                xq[:], xf[:], rs[:, 0:1], ap_sb[:, 1:2], OP.mult, OP.add
            )
            nc.vector.tensor_scalar_sub(xiT[:, kt, :], xq[:], ap_sb[:, 1:2])

        wq_sb = wpool.tile([128, O], U8)
        nc.sync.dma_start(wq_sb[:], wqT[kt * 128 : (kt + 1) * 128, :])
        if wsub_engine == "split":
            # halve the per-tile weight-dequant latency by giving one half
            # to the otherwise-idle GpSimd engine (numerically identical)
            h = O // 2
            nc.vector.tensor_sub(wiT[:, kt, :h], wq_sb[:, :h], wzpb_sb[:, :h])
            nc.gpsimd.tensor_sub(wiT[:, kt, h:], wq_sb[:, h:], wzpb_sb[:, h:])
        else:
            getattr(nc, wsub_engine).tensor_sub(wiT[:, kt, :], wq_sb[:], wzpb_sb[:])

    # --- matmul + epilogue -------------------------------------------
    for ot in range(OT):
        psums = [
            psum_pool.tile([128, 512], F32, name=f"psum_{ot}_{nb}", tag="psum")
            for nb in range(NB)
        ]
        for kt in range(KT):
            for nb in range(NB):
                nc.tensor.matmul(
                    psums[nb][:],
                    wiT[:, kt, ot * 128 : (ot + 1) * 128],
                    xiT[:, kt, nb * 512 : (nb + 1) * 512],
                    start=(kt == 0),
                    stop=(kt == KT - 1),
                )
        for nb in range(NB):
            osb = opool.tile([128, 512], F32)
            nc.scalar.activation(
                osb[:], psums[nb][:], AF.Identity,
                bias=beta[:, ot : ot + 1], scale=alpha[:, ot : ot + 1],
            )
            nc.sync.dma_start(
                out[ot * 128 : (ot + 1) * 128, nb * 512 : (nb + 1) * 512],
                osb[:],
            )


def prep_core_inputs(x, a_scale, a_zp, weight_q, w_scale, w_zp, bias):
    """Host-side sharding/layout: returns the per-core input maps."""
    x = np.asarray(x, dtype=np.float32)
    ntok = x.size // x.shape[-1]
    K = x.shape[-1]
    O_total = weight_q.shape[0]
    Oc = O_total // O_SPLIT
    Nc = ntok // N_SPLIT
    OTc = Oc // 128

    xT = np.ascontiguousarray(x.reshape(ntok, K).T)  # [K, ntok]
    s = np.float32(np.asarray(a_scale).reshape(-1)[0])
    z = np.float32(np.asarray(a_zp).reshape(-1)[0])
    aparams = np.ascontiguousarray(
        np.broadcast_to(np.array([s, z], np.float32), (128, 2))
    )

    x_halves = [
        np.ascontiguousarray(xT[:, i * Nc : (i + 1) * Nc]) for i in range(N_SPLIT)
    ]

    in_maps = []
    for c in range(O_SPLIT * N_SPLIT):
        oc, ncs = divmod(c, N_SPLIT)
        osl = slice(oc * Oc, (oc + 1) * Oc)
        wq_sh = np.asarray(weight_q[osl], dtype=np.uint8)  # values 0..255, lossless
        wqT = np.ascontiguousarray(wq_sh.T)  # [K, Oc]
        wzp_sh = np.asarray(w_zp[osl], dtype=np.float32).astype(ml_dtypes.bfloat16)
        wzpb = np.ascontiguousarray(np.broadcast_to(wzp_sh[None, :], (128, Oc)))
        wsc = np.ascontiguousarray(
            np.asarray(w_scale[osl], np.float32).reshape(OTc, 128).T
        )
        biasc = np.ascontiguousarray(
            np.asarray(bias[osl], np.float32).reshape(OTc, 128).T
        )
        in_maps.append(
            {
                "xT": x_halves[ncs],
                "wqT": wqT,
                "wzpb": wzpb,
                "aparams": aparams,
                "wsc": wsc,
                "biasc": biasc,
            }
        )
    return in_maps


_NC_CACHE = {}

# configuration validated on HW (magic-number rounding, weight-sub on DVE)
QUANT_MODE = "magic"
# "split" halves the per-k-tile weight-dequant latency (DVE + idle GpSimd),
# removing the lead-phase PE stalls the timeline model identified; it is
# numerically identical to the single-engine variant.
WSUB_ENGINE = "split"


def _get_nc(K, N, O):
    key = (K, N, O, QUANT_MODE, WSUB_ENGINE)
    if key not in _NC_CACHE:
        _NC_CACHE[key] = build_nc(K, N, O, quant=QUANT_MODE, wsub_engine=WSUB_ENGINE)
    return _NC_CACHE[key]


def kernel(x, a_scale, a_zp, weight_q, w_scale, w_zp, bias):
    x = np.asarray(x)
    b, seq, K = x.shape
    ntok = b * seq
    O_total = weight_q.shape[0]
    Oc = O_total // O_SPLIT
    Nc = ntok // N_SPLIT

    nc = _get_nc(K, Nc, Oc)
    in_maps = prep_core_inputs(x, a_scale, a_zp, weight_q, w_scale, w_zp, bias)
    res = run_bass_kernel_spmd(nc, in_maps, list(range(N_CORES)))

    outT = np.empty((O_total, ntok), np.float32)
    for c in range(N_CORES):
        oc, ncs = divmod(c, N_SPLIT)
        outT[oc * Oc : (oc + 1) * Oc, ncs * Nc : (ncs + 1) * Nc] = res.results[c]["out"]
    return np.ascontiguousarray(outT.T).reshape(b, seq, O_total)



# revision 2
# speedup vs baseline: 1.0578x; 1.0578x over previous
"""Trainium2 Bass kernel for nn_ExportPreQuantizedLayer.

Computes: out = fake_quant(x) @ dequant(weight_q).T + bias
  x_q  = clip(round_half_away(x / a_scale) + a_zp, 0, 255)
  x_dq = (x_q - a_zp) * a_scale
  W    = (weight_q - w_zp[:, None]) * w_scale[:, None]      # [out, in]
  out  = einsum('bsk,ok->bso', x_dq, W) + bias

Sharding: 2D grid over the 8 cores — 4 shards of out_features (O) x
2 shards of tokens (N).  Each core computes a [2048, 2048] block of
out^T.  Key algebra: with xi = x_q - a_zp and wi = wq - wzp (both exact
small integers, representable in bf16),

  out[o, n] = s * ws[o] * (wi @ xi^T)[o, n] + bias[o]

so the matmul runs at full bf16 PE rate and the epilogue is a single
per-partition (per-o) scale+bias.

Schedule (v2): the x fake-quant uses the HW's round+saturate u8 cast
(one DVE op) followed by a zp-subtract to bf16 (ACT in the lead phase,
DVE later), instead of the fp32 magic-number trick.  x is produced in
two N-halves: all 16 k-tiles of cols 0-1023 first (interleaved with the
weight tiles), then cols 1024-2047.  The matmul loop walks (n-half,
ot) groups of two 512-wide PSUM banks, so four output groups are in
flight in the 8 PSUM banks and the PE starts consuming k-tiles as they
are quantized; with the half-sized lead tiles the producer cadence
(~2.2us DMA) roughly matches the PE's per-k-tile appetite, instead of
starving it 2x as the full-width lead did.  Outputs leave through the
Activation HWDGE ring so stores never queue behind input loads.
"""

import sys

if "/opt/trn_rl_repo" not in sys.path:
    sys.path.insert(0, "/opt/trn_rl_repo")

import ml_dtypes
import numpy as np

import concourse.bass as bass
import concourse.mybir as mybir
import concourse.tile as tile
from concourse import bacc
from concourse.bass_utils import run_bass_kernel_spmd

F32 = mybir.dt.float32
BF16 = mybir.dt.bfloat16
U8 = mybir.dt.uint8

# Full problem shape (hardcoded per spec)
B, S, DIN, DOUT = 2, 2048, 2048, 8192
N_CORES = 8
O_SPLIT, N_SPLIT = 4, 2  # 4 shards of DOUT x 2 shards of tokens

WSUB_POOL_COLS = 768  # weight-dequant cols on GpSimd (rest on DVE)


def build_nc(K, N, O, reps=1, quant="cast", wsub_engine="split", **_):
    """Build the per-core Bass program.

    reps > 1 wraps the whole body in a device-side loop — used only for
    timing (device work scales with reps while dispatch overhead doesn't).

    Inputs (per core):
      xT      [K, N]   f32   x^T shard (tokens on the free axis)
      wqT     [K, O]   u8    weight_q^T shard
      wzpb    [128, O] bf16  w_zp broadcast along partitions
      aparams [128, 2] f32   (a_scale, a_zp) broadcast along partitions
      wsc     [128, O//128] f32  w_scale laid out [p, ot] with o = ot*128+p
      biasc   [128, O//128] f32  bias, same layout
    Output:
      out     [O, N]  f32    out^T shard
    """
    KT = K // 128
    OT = O // 128
    NH = N // 2

    nc = bacc.Bacc("TRN2", target_bir_lowering=False, debug=False, num_devices=N_CORES)
    xT = nc.declare_dram_parameter("xT", [K, N], F32, isOutput=False)
    wqT = nc.declare_dram_parameter("wqT", [K, O], U8, isOutput=False)
    wzpb = nc.declare_dram_parameter("wzpb", [128, O], BF16, isOutput=False)
    aparams = nc.declare_dram_parameter("aparams", [128, 2], F32, isOutput=False)
    wsc = nc.declare_dram_parameter("wsc", [128, OT], F32, isOutput=False)
    biasc = nc.declare_dram_parameter("biasc", [128, OT], F32, isOutput=False)
    out = nc.declare_dram_parameter("out", [O, N], F32, isOutput=True)

    with tile.TileContext(nc) as tc:
        with (
            tc.tile_pool(name="const", bufs=1) as cpool,
            tc.tile_pool(name="big", bufs=1) as bigpool,
            tc.tile_pool(name="xin", bufs=3) as xpool,
            tc.tile_pool(name="tq", bufs=3) as tpool,
            tc.tile_pool(name="win", bufs=2) as wpool,
            tc.tile_pool(name="oout", bufs=4) as opool,
            tc.tile_pool(name="psum", bufs=8, space="PSUM") as psum_pool,
        ):
            def body():
                _kernel_body(
                    nc, tc, KT, OT, NH, N, O,
                    xT, wqT, wzpb, aparams, wsc, biasc, out,
                    cpool, bigpool, xpool, tpool, wpool, opool, psum_pool,
                )

            if reps > 1:
                with tc.For_i(0, reps, 1):
                    body()
            else:
                body()

    nc.compile()
    return nc


def _kernel_body(
    nc, tc, KT, OT, NH, N, O,
    xT, wqT, wzpb, aparams, wsc, biasc, out,
    cpool, bigpool, xpool, tpool, wpool, opool, psum_pool,
):
    AF = mybir.ActivationFunctionType
    OP = mybir.AluOpType
    # --- scalar prep -------------------------------------------------
    ap_sb = cpool.tile([128, 2], F32)
    nc.sync.dma_start(ap_sb[:], aparams[:])
    rs = cpool.tile([128, 1], F32)
    nc.vector.reciprocal(rs[:], ap_sb[:, 0:1])
    negz = cpool.tile([128, 1], F32)
    nc.vector.tensor_scalar(negz[:], ap_sb[:, 1:2], -1.0, 0.0, OP.mult, OP.add)

    ws_sb = cpool.tile([128, OT], F32)
    nc.sync.dma_start(ws_sb[:], wsc[:])
    beta = cpool.tile([128, OT], F32)
    nc.sync.dma_start(beta[:], biasc[:])
    alpha = cpool.tile([128, OT], F32)  # alpha = a_scale * w_scale
    nc.vector.tensor_scalar_mul(alpha[:], ws_sb[:], ap_sb[:, 0:1])

    wzpb_sb = cpool.tile([128, O], BF16)
    nc.sync.dma_start(wzpb_sb[:, 0 : O // 2], wzpb[:, 0 : O // 2])
    nc.sync.dma_start(wzpb_sb[:, O // 2 :], wzpb[:, O // 2 :])

    # --- streaming quantization of x and weight dequant --------------
    xiT = bigpool.tile([128, KT, N], BF16)
    wiT = bigpool.tile([128, KT, O], BF16)
    dcol = O - WSUB_POOL_COLS

    # n-half 0 of x, interleaved with the weights
    for kt in range(KT):
        ks = slice(kt * 128, (kt + 1) * 128)
        wq_sb = wpool.tile([128, O], U8)
        nc.sync.dma_start(wq_sb[:], wqT[ks, :])
        xf = xpool.tile([128, NH], F32)
        nc.sync.dma_start(xf[:], xT[ks, 0:NH])
        # xq = sat_u8(round(x*rs + z)); HW cast rounds-to-nearest and
        # saturates at [0, 255], matching clip(round(x/s) + z, 0, 255)
        xq = tpool.tile([128, NH], U8)
        nc.vector.tensor_scalar(xq[:], xf[:], rs[:, 0:1], ap_sb[:, 1:2], OP.mult, OP.add)
        nc.scalar.activation(xiT[:, kt, 0:NH], xq[:], AF.Identity, bias=negz[:, 0:1])
        # weight dequant split across DVE and the otherwise-idle GpSimd
        nc.vector.tensor_sub(wiT[:, kt, :dcol], wq_sb[:, :dcol], wzpb_sb[:, :dcol])
        nc.gpsimd.tensor_sub(wiT[:, kt, dcol:], wq_sb[:, dcol:], wzpb_sb[:, dcol:])

    # n-half 1 of x (zp-subtract on DVE: ACT is busy with epilogues now)
    for kt in range(KT):
        ks = slice(kt * 128, (kt + 1) * 128)
        xf = xpool.tile([128, NH], F32)
        nc.sync.dma_start(xf[:], xT[ks, NH:N])
        xq = tpool.tile([128, NH], U8)
        nc.vector.tensor_scalar(xq[:], xf[:], rs[:, 0:1], ap_sb[:, 1:2], OP.mult, OP.add)
        nc.vector.tensor_scalar_add(xiT[:, kt, NH:N], xq[:], negz[:, 0:1])

    # --- matmul + epilogue -------------------------------------------
    for nh in range(2):
        for ot in range(OT):
            ps = [
                psum_pool.tile([128, 512], F32, name=f"ps_{nh}_{ot}_{j}", tag="psum")
                for j in range(2)
            ]
            for kt in range(KT):
                for j in range(2):
                    nc.tensor.matmul(
                        ps[j][:],
                        wiT[:, kt, ot * 128 : (ot + 1) * 128],
                        xiT[:, kt, nh * NH + j * 512 : nh * NH + (j + 1) * 512],
                        start=(kt == 0),
                        stop=(kt == KT - 1),
                    )
            osb = opool.tile([128, NH], F32)
            nc.scalar.activation(
                osb[:, 0:512], ps[0][:], AF.Identity,
                bias=beta[:, ot : ot + 1], scale=alpha[:, ot : ot + 1],
            )
            nc.vector.tensor_scalar(
                osb[:, 512:1024], ps[1][:],
                alpha[:, ot : ot + 1], beta[:, ot : ot + 1], OP.mult, OP.add,
            )
            # outputs leave via the Act HWDGE ring; input loads own the SP ring
            nc.scalar.dma_start(
                out[ot * 128 : (ot + 1) * 128, nh * NH : (nh + 1) * NH], osb[:]
            )


def prep_core_inputs(x, a_scale, a_zp, weight_q, w_scale, w_zp, bias):
    """Host-side sharding/layout: returns the per-core input maps."""
    x = np.asarray(x, dtype=np.float32)
    ntok = x.size // x.shape[-1]
    K = x.shape[-1]
    O_total = weight_q.shape[0]
    Oc = O_total // O_SPLIT
    Nc = ntok // N_SPLIT
    OTc = Oc // 128

    xT = np.ascontiguousarray(x.reshape(ntok, K).T)  # [K, ntok]
    s = np.float32(np.asarray(a_scale).reshape(-1)[0])
    z = np.float32(np.asarray(a_zp).reshape(-1)[0])
    aparams = np.ascontiguousarray(
        np.broadcast_to(np.array([s, z], np.float32), (128, 2))
    )

    x_halves = [
        np.ascontiguousarray(xT[:, i * Nc : (i + 1) * Nc]) for i in range(N_SPLIT)
    ]

    in_maps = []
    for c in range(O_SPLIT * N_SPLIT):
        oc, ncs = divmod(c, N_SPLIT)
        osl = slice(oc * Oc, (oc + 1) * Oc)
        wq_sh = np.asarray(weight_q[osl], dtype=np.uint8)  # values 0..255, lossless
        wqT = np.ascontiguousarray(wq_sh.T)  # [K, Oc]
        wzp_sh = np.asarray(w_zp[osl], dtype=np.float32).astype(ml_dtypes.bfloat16)
        wzpb = np.ascontiguousarray(np.broadcast_to(wzp_sh[None, :], (128, Oc)))
        wsc = np.ascontiguousarray(
            np.asarray(w_scale[osl], np.float32).reshape(OTc, 128).T
        )
        biasc = np.ascontiguousarray(
            np.asarray(bias[osl], np.float32).reshape(OTc, 128).T
        )
        in_maps.append(
            {
                "xT": x_halves[ncs],
                "wqT": wqT,
                "wzpb": wzpb,
                "aparams": aparams,
                "wsc": wsc,
                "biasc": biasc,
            }
        )
    return in_maps


_NC_CACHE = {}

QUANT_MODE = "cast"
WSUB_ENGINE = "split"


def _get_nc(K, N, O):
    key = (K, N, O)
    if key not in _NC_CACHE:
        _NC_CACHE[key] = build_nc(K, N, O)
    return _NC_CACHE[key]


def kernel(x, a_scale, a_zp, weight_q, w_scale, w_zp, bias):
    x = np.asarray(x)
    b, seq, K = x.shape
    ntok = b * seq
    O_total = weight_q.shape[0]
    Oc = O_total // O_SPLIT
    Nc = ntok // N_SPLIT

    nc = _get_nc(K, Nc, Oc)
    in_maps = prep_core_inputs(x, a_scale, a_zp, weight_q, w_scale, w_zp, bias)
    res = run_bass_kernel_spmd(nc, in_maps, list(range(N_CORES)))

    outT = np.empty((O_total, ntok), np.float32)
    for c in range(N_CORES):
        oc, ncs = divmod(c, N_SPLIT)
        outT[oc * Oc : (oc + 1) * Oc, ncs * Nc : (ncs + 1) * Nc] = res.results[c]["out"]
    return np.ascontiguousarray(outT.T).reshape(b, seq, O_total)


# revision 3
# speedup vs baseline: 1.1694x; 1.1056x over previous
"""Trainium2 Bass kernel for nn_ExportPreQuantizedLayer.

Computes: out = fake_quant(x) @ dequant(weight_q).T + bias
  x_q  = clip(round_half_away(x / a_scale) + a_zp, 0, 255)
  x_dq = (x_q - a_zp) * a_scale
  W    = (weight_q - w_zp[:, None]) * w_scale[:, None]      # [out, in]
  out  = einsum('bsk,ok->bso', x_dq, W) + bias

Sharding: 2D grid over the 8 cores — 4 shards of out_features (O) x
2 shards of tokens (N).  Each core computes a [2048, 2048] block of
out^T.  Key algebra: with xi = x_q - a_zp and wi = wq - wzp (both exact
small integers, representable in bf16),

  out[o, n] = s * ws[o] * (wi @ xi^T)[o, n] + bias[o]

so the matmul runs at full bf16 PE rate and the epilogue is a single
per-partition (per-o) scale+bias.

Schedule (v2): the x fake-quant uses the HW's round+saturate u8 cast
(one DVE op) followed by a zp-subtract to bf16 (ACT in the lead phase,
DVE later), instead of the fp32 magic-number trick.  x is produced in
two N-halves: all 16 k-tiles of cols 0-1023 first (interleaved with the
weight tiles), then cols 1024-2047.  The matmul loop walks (n-half,
ot) groups of two 512-wide PSUM banks, so four output groups are in
flight in the 8 PSUM banks and the PE starts consuming k-tiles as they
are quantized; with the half-sized lead tiles the producer cadence
(~2.2us DMA) roughly matches the PE's per-k-tile appetite, instead of
starving it 2x as the full-width lead did.  Outputs leave through the
Activation HWDGE ring so stores never queue behind input loads.
"""

import sys

if "/opt/trn_rl_repo" not in sys.path:
    sys.path.insert(0, "/opt/trn_rl_repo")

import ml_dtypes
import numpy as np

import concourse.bass as bass
import concourse.mybir as mybir
import concourse.tile as tile
from concourse import bacc
from concourse.bass_utils import run_bass_kernel_spmd

F32 = mybir.dt.float32
BF16 = mybir.dt.bfloat16
U8 = mybir.dt.uint8

# Full problem shape (hardcoded per spec)
B, S, DIN, DOUT = 2, 2048, 2048, 8192
N_CORES = 8
O_SPLIT, N_SPLIT = 4, 2  # 4 shards of DOUT x 2 shards of tokens

WSUB_POOL_COLS = 768  # weight-dequant cols on GpSimd (rest on DVE)


def build_nc(K, N, O, reps=1, quant="cast", wsub_engine="split", **_):
    """Build the per-core Bass program.

    reps > 1 wraps the whole body in a device-side loop — used only for
    timing (device work scales with reps while dispatch overhead doesn't).

    Inputs (per core):
      xT      [K, N]   f32   x^T shard (tokens on the free axis)
      wqT     [K, O]   u8    weight_q^T shard
      wzpb    [128, O] bf16  w_zp broadcast along partitions
      aparams [128, 2] f32   (a_scale, a_zp) broadcast along partitions
      wsc     [128, O//128] f32  w_scale laid out [p, ot] with o = ot*128+p
      biasc   [128, O//128] f32  bias, same layout
    Output:
      out     [O, N]  f32    out^T shard
    """
    KT = K // 128
    OT = O // 128
    NH = N // 2

    nc = bacc.Bacc("TRN2", target_bir_lowering=False, debug=False, num_devices=N_CORES)
    xT = nc.declare_dram_parameter("xT", [K, N], F32, isOutput=False)
    wqT = nc.declare_dram_parameter("wqT", [K, O], U8, isOutput=False)
    wzpb = nc.declare_dram_parameter("wzpb", [128, O], BF16, isOutput=False)
    aparams = nc.declare_dram_parameter("aparams", [128, 2], F32, isOutput=False)
    wsc = nc.declare_dram_parameter("wsc", [128, OT], F32, isOutput=False)
    biasc = nc.declare_dram_parameter("biasc", [128, OT], F32, isOutput=False)
    out = nc.declare_dram_parameter("out", [O, N], F32, isOutput=True)

    with tile.TileContext(nc) as tc:
        with (
            tc.tile_pool(name="const", bufs=1) as cpool,
            tc.tile_pool(name="big", bufs=1) as bigpool,
            tc.tile_pool(name="xin", bufs=3) as xpool,
            tc.tile_pool(name="tq", bufs=3) as tpool,
            tc.tile_pool(name="win", bufs=2) as wpool,
            tc.tile_pool(name="oout", bufs=4) as opool,
            tc.tile_pool(name="psum", bufs=8, space="PSUM") as psum_pool,
        ):
            def body():
                _kernel_body(
                    nc, tc, KT, OT, NH, N, O,
                    xT, wqT, wzpb, aparams, wsc, biasc, out,
                    cpool, bigpool, xpool, tpool, wpool, opool, psum_pool,
                )

            if reps > 1:
                with tc.For_i(0, reps, 1):
                    body()
            else:
                body()

    nc.compile()
    return nc


def _kernel_body(
    nc, tc, KT, OT, NH, N, O,
    xT, wqT, wzpb, aparams, wsc, biasc, out,
    cpool, bigpool, xpool, tpool, wpool, opool, psum_pool,
):
    AF = mybir.ActivationFunctionType
    OP = mybir.AluOpType
    # --- scalar prep -------------------------------------------------
    ap_sb = cpool.tile([128, 2], F32)
    nc.sync.dma_start(ap_sb[:], aparams[:])
    rs = cpool.tile([128, 1], F32)
    nc.vector.reciprocal(rs[:], ap_sb[:, 0:1])
    negz = cpool.tile([128, 1], F32)
    nc.vector.tensor_scalar(negz[:], ap_sb[:, 1:2], -1.0, 0.0, OP.mult, OP.add)

    # --- streaming quantization of x and weight dequant --------------
    # HW-measured op costs drive the engine split (per [128,1024]):
    # f32->u8 quant is 2.2us on ACT / 1.6us on DVE, u8->bf16 zp-subtract is
    # ~0.16us on DVE, w-sub [128,2048] is 1.96us on DVE (a DVE/GpSimd split
    # SERIALIZES and is slower than DVE alone).  So lead1 puts the quant
    # cast on ACT and w-sub+zp-sub on DVE (~2.2us/tile each, matching the
    # ~2.2us DMA cadence); lead2 runs all-DVE while ACT drains epilogues.
    xiT = bigpool.tile([128, KT, N], BF16)
    wiT = bigpool.tile([128, KT, O], BF16)
    wzpb_sb = cpool.tile([128, O], BF16)
    ws_sb = cpool.tile([128, OT], F32)
    beta = cpool.tile([128, OT], F32)
    alpha = cpool.tile([128, OT], F32)  # alpha = a_scale * w_scale

    def quant(dst, src, engine):
        # xq = sat_u8(round(x*rs + z)): the store-path cast rounds to
        # nearest and saturates at [0,255] = clip(round(x/s) + z, 0, 255)
        xq = tpool.tile([128, src.shape[-1]], U8, name="xq", tag="xq")
        if engine == "act":
            nc.scalar.activation(xq[:], src, AF.Identity,
                                 bias=ap_sb[:, 1:2], scale=rs[:, 0:1])
        else:
            nc.vector.tensor_scalar(xq[:], src, rs[:, 0:1], ap_sb[:, 1:2],
                                    OP.mult, OP.add)
        nc.vector.tensor_scalar_add(dst, xq[:], negz[:, 0:1])

    # n-half 0 of x, interleaved with the weights
    for kt in range(KT):
        ks = slice(kt * 128, (kt + 1) * 128)
        wq_sb = wpool.tile([128, O], U8, name="wq", tag="wq")
        nc.sync.dma_start(wq_sb[:], wqT[ks, :])
        xf = xpool.tile([128, NH], F32, name="xf", tag="xf")
        nch = 2 if kt == 0 else 1
        cw = NH // nch
        for ci in range(nch):
            cs = slice(ci * cw, (ci + 1) * cw)
            nc.sync.dma_start(xf[:, cs], xT[ks, cs])
            quant(xiT[:, kt, cs], xf[:, cs], "act")
        if kt == 0:
            # constants ride behind the first tile's loads
            nc.sync.dma_start(wzpb_sb[:, 0 : O // 2], wzpb[:, 0 : O // 2])
            nc.sync.dma_start(wzpb_sb[:, O // 2 :], wzpb[:, O // 2 :])
            nc.sync.dma_start(ws_sb[:], wsc[:])
            nc.sync.dma_start(beta[:], biasc[:])
            nc.vector.tensor_scalar_mul(alpha[:], ws_sb[:], ap_sb[:, 0:1])
            nc.vector.tensor_sub(wiT[:, kt, 0:512], wq_sb[:, 0:512],
                                 wzpb_sb[:, 0:512])
            nc.vector.tensor_sub(wiT[:, kt, 512:], wq_sb[:, 512:],
                                 wzpb_sb[:, 512:])
        else:
            nc.vector.tensor_sub(wiT[:, kt, :], wq_sb[:], wzpb_sb[:])

    # n-half 1 of x (all-DVE: ACT is busy with epilogues by now)
    for kt in range(KT):
        ks = slice(kt * 128, (kt + 1) * 128)
        xf = xpool.tile([128, NH], F32, name="xf2", tag="xf2")
        nc.sync.dma_start(xf[:], xT[ks, NH:N])
        quant(xiT[:, kt, NH:N], xf[:], "dve")

    # --- matmul + epilogue -------------------------------------------
    for nh in range(2):
        for ot in range(OT):
            ps = [
                psum_pool.tile([128, 512], F32, name=f"ps_{nh}_{ot}_{j}", tag="psum")
                for j in range(2)
            ]
            for kt in range(KT):
                for j in range(2):
                    nc.tensor.matmul(
                        ps[j][:],
                        wiT[:, kt, ot * 128 : (ot + 1) * 128],
                        xiT[:, kt, nh * NH + j * 512 : nh * NH + (j + 1) * 512],
                        start=(kt == 0),
                        stop=(kt == KT - 1),
                    )
            osb = opool.tile([128, NH], F32)
            nc.scalar.activation(
                osb[:, 0:512], ps[0][:], AF.Identity,
                bias=beta[:, ot : ot + 1], scale=alpha[:, ot : ot + 1],
            )
            nc.vector.tensor_scalar(
                osb[:, 512:1024], ps[1][:],
                alpha[:, ot : ot + 1], beta[:, ot : ot + 1], OP.mult, OP.add,
            )
            # outputs leave via the Act HWDGE ring; input loads own the SP ring
            nc.scalar.dma_start(
                out[ot * 128 : (ot + 1) * 128, nh * NH : (nh + 1) * NH], osb[:]
            )


def prep_core_inputs(x, a_scale, a_zp, weight_q, w_scale, w_zp, bias):
    """Host-side sharding/layout: returns the per-core input maps."""
    x = np.asarray(x, dtype=np.float32)
    ntok = x.size // x.shape[-1]
    K = x.shape[-1]
    O_total = weight_q.shape[0]
    Oc = O_total // O_SPLIT
    Nc = ntok // N_SPLIT
    OTc = Oc // 128

    xT = np.ascontiguousarray(x.reshape(ntok, K).T)  # [K, ntok]
    s = np.float32(np.asarray(a_scale).reshape(-1)[0])
    z = np.float32(np.asarray(a_zp).reshape(-1)[0])
    aparams = np.ascontiguousarray(
        np.broadcast_to(np.array([s, z], np.float32), (128, 2))
    )

    x_halves = [
        np.ascontiguousarray(xT[:, i * Nc : (i + 1) * Nc]) for i in range(N_SPLIT)
    ]

    in_maps = []
    for c in range(O_SPLIT * N_SPLIT):
        oc, ncs = divmod(c, N_SPLIT)
        osl = slice(oc * Oc, (oc + 1) * Oc)
        wq_sh = np.asarray(weight_q[osl], dtype=np.uint8)  # values 0..255, lossless
        wqT = np.ascontiguousarray(wq_sh.T)  # [K, Oc]
        wzp_sh = np.asarray(w_zp[osl], dtype=np.float32).astype(ml_dtypes.bfloat16)
        wzpb = np.ascontiguousarray(np.broadcast_to(wzp_sh[None, :], (128, Oc)))
        wsc = np.ascontiguousarray(
            np.asarray(w_scale[osl], np.float32).reshape(OTc, 128).T
        )
        biasc = np.ascontiguousarray(
            np.asarray(bias[osl], np.float32).reshape(OTc, 128).T
        )
        in_maps.append(
            {
                "xT": x_halves[ncs],
                "wqT": wqT,
                "wzpb": wzpb,
                "aparams": aparams,
                "wsc": wsc,
                "biasc": biasc,
            }
        )
    return in_maps


_NC_CACHE = {}

QUANT_MODE = "cast"
WSUB_ENGINE = "split"


def _get_nc(K, N, O):
    key = (K, N, O)
    if key not in _NC_CACHE:
        _NC_CACHE[key] = build_nc(K, N, O)
    return _NC_CACHE[key]


def kernel(x, a_scale, a_zp, weight_q, w_scale, w_zp, bias):
    x = np.asarray(x)
    b, seq, K = x.shape
    ntok = b * seq
    O_total = weight_q.shape[0]
    Oc = O_total // O_SPLIT
    Nc = ntok // N_SPLIT

    nc = _get_nc(K, Nc, Oc)
    in_maps = prep_core_inputs(x, a_scale, a_zp, weight_q, w_scale, w_zp, bias)
    res = run_bass_kernel_spmd(nc, in_maps, list(range(N_CORES)))

    outT = np.empty((O_total, ntok), np.float32)
    for c in range(N_CORES):
        oc, ncs = divmod(c, N_SPLIT)
        outT[oc * Oc : (oc + 1) * Oc, ncs * Nc : (ncs + 1) * Nc] = res.results[c]["out"]
    return np.ascontiguousarray(outT.T).reshape(b, seq, O_total)
